# revision 23
# baseline (speedup 1.0000x reference)
"""Trainium2 Bass kernel for the sparse-attention problem — v10.

Per batch element (one NeuronCore each):
  pooled[bb, wb] = 16x16 block-sum of label rows 160:320  (bb = c*10+hb)
  lab[q] = argmax_c pooled[c*10+hb, wb],  q = hb*128+wb
  e = where(same XOR (en>0), 0.5-(en>0), en);  att = softmax(e, -1)

v10 — the two-stage software-pipeline release.  HW phase isolation showed
phase 1 (label loads + PE pooling + argmax) alone runs at 26.4 us/rep and
phase 2 (energy + mask + softmax + stores) alone at 17.3 us/rep, yet the
v3..v9 kernels all measured ~51 us: the phases executed serially because
instructions issue per-engine IN PROGRAM ORDER, so rep N's pooling
reduces/transposes sat behind rep N-1's entire phase-2 stream on the DVE,
and the PE's group-2 matmuls (PSUM WAW on the collect tile) waited for all
of it.  v10 emits a true depth-2 pipeline: iteration n interleaves
  phase 2 of rep n-1   (DVE masks + Act exp + att/z stores + prefetches)
with
  phase 1 of rep n     (SP-ring label stream + PE pooling + argmax)
chunk-by-chunk, so every engine's stream matches the overlapped schedule.

Carried over from v3..v9 (all HW-validated):
  - label fp16 with SUM-PRESERVING DITHERED rounding (zero argmax flips);
    24 natural 0.5 MB tile loads on the SP ring (~440 GB/s; the SP ring
    carries nothing else, so the stream never stalls).
  - energy/att fp16; e not stored: host decodes e = log(att) + log(Z)
    from att + the 5 KB device-computed Z row sums.
  - supertile layout: partition p <-> energy/att rows {j*128+p}, so the
    phase-2 row-label table IS the argmax output tile (lab_all) and every
    energy/att chunk transfer is a contiguous [128 x 2560B] 2D DMA (the
    fastest store shape measured); energy loads/att stores ride the Act
    ring chunk-wise, prefetching the next rep's energy + sign masks during
    the current phase 2.
  - energy tile double-buffered; all pooling on the PE (fp16 band-matmul,
    f32 PSUM accumulate); phase 2 software-pipelined across DVE/Act.
"""

import numpy as np

_CACHE: dict = {}

B = 8
C = 19
HB = 10
WB = 128
ROWS = C * HB * 16  # 3040
W = 2048
P = HB * WB  # 1280
TILE_ROWS = 128
N_LTILES = (ROWS + TILE_ROWS - 1) // TILE_ROWS  # 24
NPAIR = C * HB  # 190

STORE_E = False


def _build(
    reps: int = 1,
    scale_on_act: bool = False,
    lt_bufs: int = 10,
    mm_n: int = 512,
    store_e: bool = False,
    pipe2: bool = True,
    gp_tiles: int = 0,
    tv_isle: bool = False,
    tv_on_gp: bool = False,
    bcast2: bool = False,
):
    import concourse.bacc as bacc
    import concourse.tile as tile
    import concourse.mybir as mybir
    from concourse.mybir import AluOpType as op, ActivationFunctionType as act

    f32 = mybir.dt.float32
    f16 = mybir.dt.float16
    u16 = mybir.dt.uint16
    u32 = mybir.dt.uint32

    nc = bacc.Bacc("TRN2", target_bir_lowering=False, debug=False, num_devices=B)

    label_d = nc.dram_tensor("label", [ROWS, W], f16, kind="ExternalInput")
    energy_d = nc.dram_tensor("energy", [P, P], f16, kind="ExternalInput")
    att_d = nc.dram_tensor("att_out", [P, P], f16, kind="ExternalOutput")
    if store_e:
        e_d = nc.dram_tensor("e_out", [P, P], f16, kind="ExternalOutput")
    z_d = nc.dram_tensor("z_out", [128, HB], f32, kind="ExternalOutput")

    ident_d = nc.inline_tensor(np.eye(128, dtype=np.float32), name="ident")
    ones_d = nc.inline_tensor(np.ones((1, 128), dtype=np.float32), name="ones1")
    if bcast2:
        # onehots[k, hb*128+c] = 1 iff k == hb: stationary slices that
        # broadcast row hb of the transposed label tile to all partitions
        oh = np.zeros((16, HB * 128), dtype=np.float16)
        for hb_ in range(HB):
            oh[hb_, hb_ * 128 : (hb_ + 1) * 128] = 1.0
        oneh_d = nc.inline_tensor(oh, name="onehots")
    # band[p, j] = 1 iff j == p//16 + 120; group slot s uses
    # band[:, 120-8s : 248-8s] so tile s's 8 h-block sums land on collect
    # partitions 8s..8s+7.
    band_d = nc.dram_tensor("band", [128, 248], f16, kind="ExternalInput")

    mm_chunks = [(c0, min(c0 + mm_n, W)) for c0 in range(0, W, mm_n)]
    groups = [list(range(0, 16)), list(range(16, N_LTILES))]

    with tile.TileContext(nc) as tc:
        with (
            tc.tile_pool(name="consts", bufs=1) as consts,
            tc.tile_pool(name="sup", bufs=1) as sup,
            tc.tile_pool(name="lt", bufs=lt_bufs) as ltp,
            tc.tile_pool(name="wt", bufs=3) as wtp,
            tc.tile_pool(name="lab", bufs=1) as labp,
            tc.tile_pool(name="mx", bufs=4) as mxp,
            tc.tile_pool(name="pm", bufs=3) as pmp,
            tc.tile_pool(name="col", bufs=1, space="PSUM") as colp,
            tc.tile_pool(name="psA", bufs=2, space="PSUM") as psA,
            tc.tile_pool(name="psB", bufs=2, space="PSUM") as psB,
        ):
            ident = consts.tile([128, 128], f32, tag="ident")
            nc.sync.dma_start(ident[:], ident_d[:])
            if bcast2:
                oneh = consts.tile([16, HB * 128], f16, tag="onehots")
                nc.sync.dma_start(oneh[:], oneh_d[:])
                tplS = consts.tile([16, 128], f16, tag="tplS")
            ones1 = consts.tile([1, 128], f32, tag="ones1")
            nc.sync.dma_start(ones1[:], ones_d[:])
            band_t = consts.tile([128, 248], f16, tag="band")
            nc.sync.dma_start(band_t[:], band_d[:])

            et0 = sup.tile([128, HB * P], f16, tag="et0")
            et1 = sup.tile([128, HB * P], f16, tag="et1")
            ets = [et0, et1]
            gt = sup.tile([128, HB * P], f16, tag="gt")
            tv = sup.tile([128, HB * P], f16, tag="tv")
            att_s = sup.tile([128, HB * P], f16, tag="att")

            pooled = labp.tile([128, NPAIR], f32, tag="pooled")
            lab_all = labp.tile([128, 16], f32, tag="lab_all")
            labF = labp.tile([1, P], f32, tag="labF")
            lab_cols = labp.tile([128, P], f16, tag="lab_cols")
            sm = labp.tile([128, HB], f32, tag="sm")
            rc = labp.tile([128, HB], f32, tag="rc")

            def load_en_chunk(m, j):
                """energy rows j*128..j*128+127 -> et chunk j of rep m."""
                s = slice(j * P, (j + 1) * P)
                nc.scalar.dma_start(
                    ets[m % 2][:, s], energy_d[j * 128 : (j + 1) * 128, :]
                )

            def gt_tv_chunk(m, j):
                """sign masks for rep m's chunk j.  tv = 0.5 - (en>0) is
                computed as (en<=0) - 0.5 (identical, incl. en==0): one pass
                straight from the energy tile, independent of gt, optionally
                on the otherwise-idle GPSIMD engine."""
                s = slice(j * P, (j + 1) * P)
                src = ets[m % 2]
                nc.vector.tensor_scalar(gt[:, s], src[:, s], 0.0, None, op.is_gt)
                if tv_isle:
                    eng = nc.gpsimd if tv_on_gp else nc.vector
                    eng.tensor_scalar(
                        tv[:, s], src[:, s], 0.0, -0.5, op.is_le, op.add
                    )
                else:
                    nc.vector.tensor_scalar(
                        tv[:, s], gt[:, s], -1.0, 0.5, op.mult, op.add
                    )

            def ph2_chunk(m, j, lag):
                """phase-2 chunk j of rep m (deferred scale by `lag`), plus
                next-rep prefetches."""
                et = ets[m % 2]
                if j < HB:
                    s = slice(j * P, (j + 1) * P)
                    pm = pmp.tile([128, P], u16, tag="pm")
                    nc.vector.scalar_tensor_tensor(
                        pm[:],
                        lab_cols[:],
                        lab_all[:, j : j + 1],
                        gt[:, s],
                        op0=op.is_equal,
                        op1=op.logical_xor,
                    )
                    nc.vector.copy_predicated(et[:, s], pm[:], tv[:, s])
                    if store_e:
                        nc.scalar.dma_start(
                            e_d[j * 128 : (j + 1) * 128, :], et[:, s]
                        )
                    nc.scalar.activation(
                        att_s[:, s], et[:, s], act.Exp,
                        accum_out=sm[:, j : j + 1],
                    )
                i = j - lag
                if 0 <= i < HB:
                    si = slice(i * P, (i + 1) * P)
                    nc.vector.reciprocal(rc[:, i : i + 1], sm[:, i : i + 1])
                    if scale_on_act:
                        nc.scalar.activation(
                            att_s[:, si], att_s[:, si], act.Copy, bias=0.0,
                            scale=rc[:, i : i + 1],
                        )
                    else:
                        nc.vector.tensor_scalar(
                            att_s[:, si], att_s[:, si], rc[:, i : i + 1],
                            None, op.mult,
                        )
                    nc.scalar.dma_start(
                        att_d[i * 128 : (i + 1) * 128, :], att_s[:, si]
                    )
                    if m + 1 < reps:
                        # prefetch rep m+1's energy chunk + sign masks
                        load_en_chunk(m + 1, i)
                        gt_tv_chunk(m + 1, i)

            lag = 1 if pipe2 else 0
            NCH = HB + lag  # phase-2 chunk iterations per rep

            # ---------- depth-2 pipeline over iterations n ----------------
            # iter n: phase 1 of rep n  +  phase 2 of rep n-1 (interleaved)
            for n in range(reps + 1):
                do1 = n < reps
                do2 = n > 0
                m = n - 1  # the rep whose phase 2 runs this iteration

                if n == 0:
                    # prologue: rep 0's energy + sign masks
                    for jj in range(HB):
                        load_en_chunk(0, jj)
                    for jj in range(HB):
                        gt_tv_chunk(0, jj)

                # phase-2 chunks are spread across the phase-1 emission so
                # each engine's stream matches the overlapped schedule.
                ch = 0

                def emit_ph2(upto):
                    nonlocal ch
                    while do2 and ch < upto:
                        ph2_chunk(m, ch, lag)
                        ch += 1

                for tiles in (groups if do1 else []):
                    g0 = tiles[0]
                    col = colp.tile([128, W], f32, tag="col")
                    for t in tiles:
                        r0 = t * TILE_ROWS
                        nr = min(TILE_ROWS, ROWS - r0)
                        lt = ltp.tile([128, W], f16, tag="lt")
                        # optionally route every 4th tile over the gpsimd
                        # SWDGE ring to widen the label-load path
                        use_gp = gp_tiles > 0 and (t % 4 == 3) and (
                            t // 4 < gp_tiles
                        )
                        (nc.gpsimd if use_gp else nc.sync).dma_start(
                            lt[:nr, :], label_d[r0 : r0 + nr, :]
                        )
                        s = t - g0
                        off = 120 - 8 * s
                        first = t == tiles[0]
                        last = t == tiles[-1]
                        for c0, c1 in mm_chunks:
                            nc.tensor.matmul(
                                col[:, c0:c1],
                                band_t[:nr, off : off + 128],
                                lt[:nr, c0:c1],
                                start=first,
                                stop=last,
                                skip_group_check=True,
                            )
                        # ~one phase-2 chunk interleaved per two label tiles
                        emit_ph2(min((t + 1) * NCH // N_LTILES, NCH))
                    nbb = 8 * (len(tiles) - 1) + (
                        min(TILE_ROWS, ROWS - tiles[-1] * TILE_ROWS) // 16
                    )
                    pT = wtp.tile([128, 128], f32, tag="pT")
                    nc.vector.tensor_reduce(
                        pT[:nbb, :],
                        col[:nbb, :].rearrange("p (c w) -> p c w", w=16),
                        axis=mybir.AxisListType.X,
                        op=op.add,
                    )
                    tp = psA.tile([128, 128], f32, tag="tp")
                    nc.tensor.transpose(tp[:, :nbb], pT[:nbb, :], ident[:nbb, :nbb])
                    nc.scalar.copy(pooled[:, 8 * g0 : 8 * g0 + nbb], tp[:, :nbb])
                # drain remaining phase-2 chunks (this is all of them when
                # do1 is False, i.e. the epilogue iteration)
                emit_ph2(NCH)
                if do2:
                    nc.scalar.dma_start(z_d[:, :], sm[:, :])

                if not do1:
                    break

                # ---- labels: argmax over c per position ------------------
                pooled_v = pooled[:, :NPAIR].rearrange("p (c h) -> p h c", h=HB)
                for hb in range(HB):
                    vals = pooled_v[:, hb, :]
                    mx = mxp.tile([128, 8], f32, tag="mx")
                    nc.vector.max(mx[:], vals)
                    idx = mxp.tile([128, 8], u32, tag="idx")
                    nc.vector.max_index(idx[:], mx[:], vals)
                    nc.vector.tensor_copy(lab_all[:, hb : hb + 1], idx[:, 0:1])
                if bcast2:
                    # one transpose + 10 one-hot broadcast matmuls:
                    # lab_cols[p, hb*128+wb] = lab_all[wb, hb] for all p
                    tpl = psA.tile([128, 128], f32, tag="tp")
                    nc.tensor.transpose(
                        tpl[:16, :], lab_all[:, :16], ident[:, :]
                    )
                    nc.scalar.copy(tplS[:HB, :], tpl[:HB, :])
                    for hb in range(HB):
                        n0 = hb * 128
                        bb = psB.tile([128, 512], f32, tag="bb")
                        nc.tensor.matmul(
                            bb[:, :128],
                            oneh[:HB, n0 : n0 + 128],
                            tplS[:HB, :],
                        )
                        nc.scalar.copy(lab_cols[:, n0 : n0 + 128], bb[:, :128])
                else:
                    # labF[0, hb*128+wb] = lab_all[wb, hb]; lab_cols = bcast
                    for hb in range(HB):
                        tpl = psA.tile([128, 128], f32, tag="tp")
                        nc.tensor.transpose(tpl[0:1, :], lab_all[:, hb : hb + 1], ident[:, :])
                        nc.scalar.copy(labF[0:1, hb * 128 : (hb + 1) * 128], tpl[0:1, :])
                    for j in range(3):
                        n0 = j * 512
                        n1 = min(P, n0 + 512)
                        bb = psB.tile([128, 512], f32, tag="bb")
                        nc.tensor.matmul(bb[:, : n1 - n0], ones1[:, :], labF[0:1, n0:n1])
                        nc.scalar.copy(lab_cols[:, n0:n1], bb[:, : n1 - n0])

    nc.compile()
    return nc


def _get_nc():
    if "nc" not in _CACHE:
        _CACHE["nc"] = _build(store_e=_CACHE.get("store_e", STORE_E))
    return _CACHE["nc"]


def band_array() -> np.ndarray:
    band = np.zeros((128, 248), dtype=np.float16)
    for p_ in range(128):
        band[p_, p_ // 16 + 120] = 1.0
    return band


def dither_label_fp16(lab_slice: np.ndarray) -> np.ndarray:
    """Round [ROWS, W] f32 label data to fp16 so that every 16x16 block sum
    is preserved to ~1e-5: round-nearest everywhere, then re-round the
    smallest-|x| element per block to absorb the block's rounding error."""
    x = (
        lab_slice.reshape(NPAIR, 16, WB, 16)
        .transpose(0, 2, 1, 3)
        .reshape(-1, 256)
        .astype(np.float64)
    )
    q = x.astype(np.float16)
    r = x - q.astype(np.float64)
    m = np.abs(x).argmin(axis=1)
    rows = np.arange(x.shape[0])
    E = r.sum(axis=1) - r[rows, m]
    q[rows, m] = (x[rows, m] + E).astype(np.float16)
    return (
        q.reshape(NPAIR, WB, 16, 16).transpose(0, 2, 1, 3).reshape(ROWS, W)
    )


def kernel(label: np.ndarray, energy: np.ndarray):
    from concourse import bass_utils

    store_e = _CACHE.get("store_e", STORE_E)
    nc = _get_nc()
    band = band_array()
    in_maps = []
    for i in range(B):
        lab_i = dither_label_fp16(
            np.ascontiguousarray(label[i, :, 160:320, :], dtype=np.float32).reshape(
                ROWS, W
            )
        )
        en_i = np.ascontiguousarray(energy[i]).astype(np.float16)
        in_maps.append({"label": lab_i, "energy": en_i, "band": band})

    res = bass_utils.run_bass_kernel_spmd(nc, in_maps, core_ids=list(range(B)))
    _CACHE["last_result"] = res

    att = np.stack([res.results[i]["att_out"].astype(np.float32) for i in range(B)])
    if store_e:
        e = np.stack([res.results[i]["e_out"].astype(np.float32) for i in range(B)])
    else:
        # z[p, j] = Z of row j*128+p
        z = np.stack([res.results[i]["z_out"].T.reshape(P) for i in range(B)])
        e = np.log(np.maximum(att, 1e-30)) + np.log(z)[:, :, None]
    return e, att


# revision 24
# speedup vs baseline: 1.4959x; 1.4959x over previous
"""Trainium2 Bass kernel for the sparse-attention problem — v10.

Per batch element (one NeuronCore each):
  pooled[bb, wb] = 16x16 block-sum of label rows 160:320  (bb = c*10+hb)
  lab[q] = argmax_c pooled[c*10+hb, wb],  q = hb*128+wb
  e = where(same XOR (en>0), 0.5-(en>0), en);  att = softmax(e, -1)

v10 — the two-stage software-pipeline release.  HW phase isolation showed
phase 1 (label loads + PE pooling + argmax) alone runs at 26.4 us/rep and
phase 2 (energy + mask + softmax + stores) alone at 17.3 us/rep, yet the
v3..v9 kernels all measured ~51 us: the phases executed serially because
instructions issue per-engine IN PROGRAM ORDER, so rep N's pooling
reduces/transposes sat behind rep N-1's entire phase-2 stream on the DVE,
and the PE's group-2 matmuls (PSUM WAW on the collect tile) waited for all
of it.  v10 emits a true depth-2 pipeline: iteration n interleaves
  phase 2 of rep n-1   (DVE masks + Act exp + att/z stores + prefetches)
with
  phase 1 of rep n     (SP-ring label stream + PE pooling + argmax)
chunk-by-chunk, so every engine's stream matches the overlapped schedule.

Carried over from v3..v9 (all HW-validated):
  - label fp16 with SUM-PRESERVING DITHERED rounding (zero argmax flips);
    24 natural 0.5 MB tile loads on the SP ring (~440 GB/s; the SP ring
    carries nothing else, so the stream never stalls).
  - energy/att fp16; e not stored: host decodes e = log(att) + log(Z)
    from att + the 5 KB device-computed Z row sums.
  - supertile layout: partition p <-> energy/att rows {j*128+p}, so the
    phase-2 row-label table IS the argmax output tile (lab_all) and every
    energy/att chunk transfer is a contiguous [128 x 2560B] 2D DMA (the
    fastest store shape measured); energy loads/att stores ride the Act
    ring chunk-wise, prefetching the next rep's energy + sign masks during
    the current phase 2.
  - energy tile double-buffered; all pooling on the PE (fp16 band-matmul,
    f32 PSUM accumulate); phase 2 software-pipelined across DVE/Act.
"""

import numpy as np

_CACHE: dict = {}

B = 8
C = 19
HB = 10
WB = 128
ROWS = C * HB * 16  # 3040
W = 2048
P = HB * WB  # 1280
TILE_ROWS = 128
N_LTILES = (ROWS + TILE_ROWS - 1) // TILE_ROWS  # 24
NPAIR = C * HB  # 190

STORE_E = False


def _build(
    reps: int = 1,
    scale_on_act: bool = False,
    lt_bufs: int = 10,
    mm_n: int = 512,
    store_e: bool = False,
    pipe2: bool = True,
    gp_tiles: int = 0,
    tv_isle: bool = False,
    tv_on_gp: bool = False,
    bcast2: bool = False,
    gp_odd: bool = False,
):
    import concourse.bacc as bacc
    import concourse.tile as tile
    import concourse.mybir as mybir
    from concourse.mybir import AluOpType as op, ActivationFunctionType as act

    f32 = mybir.dt.float32
    f16 = mybir.dt.float16
    u16 = mybir.dt.uint16
    u32 = mybir.dt.uint32

    nc = bacc.Bacc("TRN2", target_bir_lowering=False, debug=False, num_devices=B)

    label_d = nc.dram_tensor("label", [ROWS, W], f16, kind="ExternalInput")
    energy_d = nc.dram_tensor("energy", [P, P], f16, kind="ExternalInput")
    att_d = nc.dram_tensor("att_out", [P, P], f16, kind="ExternalOutput")
    if store_e:
        e_d = nc.dram_tensor("e_out", [P, P], f16, kind="ExternalOutput")
    z_d = nc.dram_tensor("z_out", [128, HB], f32, kind="ExternalOutput")

    ident_d = nc.inline_tensor(np.eye(128, dtype=np.float32), name="ident")
    ones_d = nc.inline_tensor(np.ones((1, 128), dtype=np.float32), name="ones1")
    if bcast2:
        # onehots[k, hb*128+c] = 1 iff k == hb: stationary slices that
        # broadcast row hb of the transposed label tile to all partitions
        oh = np.zeros((16, HB * 128), dtype=np.float16)
        for hb_ in range(HB):
            oh[hb_, hb_ * 128 : (hb_ + 1) * 128] = 1.0
        oneh_d = nc.inline_tensor(oh, name="onehots")
    # band[p, j] = 1 iff j == p//16 + 120; group slot s uses
    # band[:, 120-8s : 248-8s] so tile s's 8 h-block sums land on collect
    # partitions 8s..8s+7.
    band_d = nc.dram_tensor("band", [128, 248], f16, kind="ExternalInput")

    mm_chunks = [(c0, min(c0 + mm_n, W)) for c0 in range(0, W, mm_n)]
    groups = [list(range(0, 16)), list(range(16, N_LTILES))]

    with tile.TileContext(nc) as tc:
        with (
            tc.tile_pool(name="consts", bufs=1) as consts,
            tc.tile_pool(name="sup", bufs=1) as sup,
            tc.tile_pool(name="lt", bufs=lt_bufs) as ltp,
            tc.tile_pool(name="wt", bufs=3) as wtp,
            tc.tile_pool(name="lab", bufs=1) as labp,
            tc.tile_pool(name="mx", bufs=4) as mxp,
            tc.tile_pool(name="pm", bufs=3) as pmp,
            tc.tile_pool(name="col", bufs=1, space="PSUM") as colp,
            tc.tile_pool(name="psA", bufs=2, space="PSUM") as psA,
            tc.tile_pool(name="psB", bufs=2, space="PSUM") as psB,
        ):
            ident = consts.tile([128, 128], f32, tag="ident")
            nc.sync.dma_start(ident[:], ident_d[:])
            if bcast2:
                oneh = consts.tile([16, HB * 128], f16, tag="onehots")
                nc.sync.dma_start(oneh[:], oneh_d[:])
                tplS = consts.tile([16, 128], f16, tag="tplS")
            ones1 = consts.tile([1, 128], f32, tag="ones1")
            nc.sync.dma_start(ones1[:], ones_d[:])
            band_t = consts.tile([128, 248], f16, tag="band")
            nc.sync.dma_start(band_t[:], band_d[:])

            et0 = sup.tile([128, HB * P], f16, tag="et0")
            et1 = sup.tile([128, HB * P], f16, tag="et1")
            ets = [et0, et1]
            gt = sup.tile([128, HB * P], f16, tag="gt")
            tv = sup.tile([128, HB * P], f16, tag="tv")
            att_s = sup.tile([128, HB * P], f16, tag="att")

            pooled = labp.tile([128, NPAIR], f32, tag="pooled")
            lab_all = labp.tile([128, 16], f32, tag="lab_all")
            labF = labp.tile([1, P], f32, tag="labF")
            lab_cols = labp.tile([128, P], f16, tag="lab_cols")
            sm = labp.tile([128, HB], f32, tag="sm")
            rc = labp.tile([128, HB], f32, tag="rc")

            def load_en_chunk(m, j):
                """energy rows j*128..j*128+127 -> et chunk j of rep m."""
                s = slice(j * P, (j + 1) * P)
                nc.scalar.dma_start(
                    ets[m % 2][:, s], energy_d[j * 128 : (j + 1) * 128, :]
                )

            def gt_tv_chunk(m, j):
                """sign masks for rep m's chunk j.  tv = 0.5 - (en>0) is
                computed as (en<=0) - 0.5 (identical, incl. en==0): one pass
                straight from the energy tile, independent of gt, optionally
                on the otherwise-idle GPSIMD engine."""
                s = slice(j * P, (j + 1) * P)
                src = ets[m % 2]
                nc.vector.tensor_scalar(gt[:, s], src[:, s], 0.0, None, op.is_gt)
                if tv_isle:
                    eng = nc.gpsimd if tv_on_gp else nc.vector
                    eng.tensor_scalar(
                        tv[:, s], src[:, s], 0.0, -0.5, op.is_le, op.add
                    )
                else:
                    nc.vector.tensor_scalar(
                        tv[:, s], gt[:, s], -1.0, 0.5, op.mult, op.add
                    )

            def ph2_chunk(m, j, lag):
                """phase-2 chunk j of rep m (deferred scale by `lag`), plus
                next-rep prefetches."""
                et = ets[m % 2]
                if j < HB:
                    s = slice(j * P, (j + 1) * P)
                    pm = pmp.tile([128, P], u16, tag="pm")
                    stt_eng = nc.gpsimd if (gp_odd and j % 2 == 1) else nc.vector
                    stt_eng.scalar_tensor_tensor(
                        pm[:],
                        lab_cols[:],
                        lab_all[:, j : j + 1],
                        gt[:, s],
                        op0=op.is_equal,
                        op1=op.logical_xor,
                    )
                    nc.vector.copy_predicated(et[:, s], pm[:], tv[:, s])
                    if store_e:
                        nc.scalar.dma_start(
                            e_d[j * 128 : (j + 1) * 128, :], et[:, s]
                        )
                    nc.scalar.activation(
                        att_s[:, s], et[:, s], act.Exp,
                        accum_out=sm[:, j : j + 1],
                    )
                i = j - lag
                if 0 <= i < HB:
                    si = slice(i * P, (i + 1) * P)
                    nc.vector.reciprocal(rc[:, i : i + 1], sm[:, i : i + 1])
                    if scale_on_act:
                        nc.scalar.activation(
                            att_s[:, si], att_s[:, si], act.Copy, bias=0.0,
                            scale=rc[:, i : i + 1],
                        )
                    else:
                        sc_eng = (
                            nc.gpsimd if (gp_odd and i % 2 == 1) else nc.vector
                        )
                        sc_eng.tensor_scalar(
                            att_s[:, si], att_s[:, si], rc[:, i : i + 1],
                            None, op.mult,
                        )
                    nc.scalar.dma_start(
                        att_d[i * 128 : (i + 1) * 128, :], att_s[:, si]
                    )
                    if m + 1 < reps:
                        # prefetch rep m+1's energy chunk + sign masks
                        load_en_chunk(m + 1, i)
                        gt_tv_chunk(m + 1, i)

            lag = 1 if pipe2 else 0
            NCH = HB + lag  # phase-2 chunk iterations per rep

            # ---------- depth-2 pipeline over iterations n ----------------
            # iter n: phase 1 of rep n  +  phase 2 of rep n-1 (interleaved)
            for n in range(reps + 1):
                do1 = n < reps
                do2 = n > 0
                m = n - 1  # the rep whose phase 2 runs this iteration

                if n == 0:
                    # prologue: rep 0's energy + sign masks
                    for jj in range(HB):
                        load_en_chunk(0, jj)
                    for jj in range(HB):
                        gt_tv_chunk(0, jj)

                # phase-2 chunks are spread across the phase-1 emission so
                # each engine's stream matches the overlapped schedule.
                ch = 0

                def emit_ph2(upto):
                    nonlocal ch
                    while do2 and ch < upto:
                        ph2_chunk(m, ch, lag)
                        ch += 1

                for tiles in (groups if do1 else []):
                    g0 = tiles[0]
                    col = colp.tile([128, W], f32, tag="col")
                    for t in tiles:
                        r0 = t * TILE_ROWS
                        nr = min(TILE_ROWS, ROWS - r0)
                        lt = ltp.tile([128, W], f16, tag="lt")
                        # optionally route every 4th tile over the gpsimd
                        # SWDGE ring to widen the label-load path
                        use_gp = gp_tiles > 0 and (t % 4 == 3) and (
                            t // 4 < gp_tiles
                        )
                        (nc.gpsimd if use_gp else nc.sync).dma_start(
                            lt[:nr, :], label_d[r0 : r0 + nr, :]
                        )
                        s = t - g0
                        off = 120 - 8 * s
                        first = t == tiles[0]
                        last = t == tiles[-1]
                        for c0, c1 in mm_chunks:
                            nc.tensor.matmul(
                                col[:, c0:c1],
                                band_t[:nr, off : off + 128],
                                lt[:nr, c0:c1],
                                start=first,
                                stop=last,
                                skip_group_check=True,
                            )
                        # ~one phase-2 chunk interleaved per two label tiles
                        emit_ph2(min((t + 1) * NCH // N_LTILES, NCH))
                    nbb = 8 * (len(tiles) - 1) + (
                        min(TILE_ROWS, ROWS - tiles[-1] * TILE_ROWS) // 16
                    )
                    pT = wtp.tile([128, 128], f32, tag="pT")
                    nc.vector.tensor_reduce(
                        pT[:nbb, :],
                        col[:nbb, :].rearrange("p (c w) -> p c w", w=16),
                        axis=mybir.AxisListType.X,
                        op=op.add,
                    )
                    tp = psA.tile([128, 128], f32, tag="tp")
                    nc.tensor.transpose(tp[:, :nbb], pT[:nbb, :], ident[:nbb, :nbb])
                    nc.scalar.copy(pooled[:, 8 * g0 : 8 * g0 + nbb], tp[:, :nbb])
                # drain remaining phase-2 chunks (this is all of them when
                # do1 is False, i.e. the epilogue iteration)
                emit_ph2(NCH)
                if do2:
                    nc.scalar.dma_start(z_d[:, :], sm[:, :])

                if not do1:
                    break

                # ---- labels: argmax over c per position ------------------
                pooled_v = pooled[:, :NPAIR].rearrange("p (c h) -> p h c", h=HB)
                for hb in range(HB):
                    vals = pooled_v[:, hb, :]
                    mx = mxp.tile([128, 8], f32, tag="mx")
                    nc.vector.max(mx[:], vals)
                    idx = mxp.tile([128, 8], u32, tag="idx")
                    nc.vector.max_index(idx[:], mx[:], vals)
                    nc.vector.tensor_copy(lab_all[:, hb : hb + 1], idx[:, 0:1])
                if bcast2:
                    # one transpose + 10 one-hot broadcast matmuls:
                    # lab_cols[p, hb*128+wb] = lab_all[wb, hb] for all p
                    tpl = psA.tile([128, 128], f32, tag="tp")
                    nc.tensor.transpose(
                        tpl[:16, :], lab_all[:, :16], ident[:, :]
                    )
                    nc.scalar.copy(tplS[:HB, :], tpl[:HB, :])
                    for hb in range(HB):
                        n0 = hb * 128
                        bb = psB.tile([128, 512], f32, tag="bb")
                        nc.tensor.matmul(
                            bb[:, :128],
                            oneh[:HB, n0 : n0 + 128],
                            tplS[:HB, :],
                        )
                        nc.scalar.copy(lab_cols[:, n0 : n0 + 128], bb[:, :128])
                else:
                    # labF[0, hb*128+wb] = lab_all[wb, hb]; lab_cols = bcast
                    for hb in range(HB):
                        tpl = psA.tile([128, 128], f32, tag="tp")
                        nc.tensor.transpose(tpl[0:1, :], lab_all[:, hb : hb + 1], ident[:, :])
                        nc.scalar.copy(labF[0:1, hb * 128 : (hb + 1) * 128], tpl[0:1, :])
                    for j in range(3):
                        n0 = j * 512
                        n1 = min(P, n0 + 512)
                        bb = psB.tile([128, 512], f32, tag="bb")
                        nc.tensor.matmul(bb[:, : n1 - n0], ones1[:, :], labF[0:1, n0:n1])
                        nc.scalar.copy(lab_cols[:, n0:n1], bb[:, : n1 - n0])

    nc.compile()
    return nc


def _get_nc():
    if "nc" not in _CACHE:
        _CACHE["nc"] = _build(store_e=_CACHE.get("store_e", STORE_E))
    return _CACHE["nc"]


def band_array() -> np.ndarray:
    band = np.zeros((128, 248), dtype=np.float16)
    for p_ in range(128):
        band[p_, p_ // 16 + 120] = 1.0
    return band


def dither_label_fp16(lab_slice: np.ndarray) -> np.ndarray:
    """Round [ROWS, W] f32 label data to fp16 so that every 16x16 block sum
    is preserved to ~1e-5: round-nearest everywhere, then re-round the
    smallest-|x| element per block to absorb the block's rounding error."""
    x = (
        lab_slice.reshape(NPAIR, 16, WB, 16)
        .transpose(0, 2, 1, 3)
        .reshape(-1, 256)
        .astype(np.float64)
    )
    q = x.astype(np.float16)
    r = x - q.astype(np.float64)
    m = np.abs(x).argmin(axis=1)
    rows = np.arange(x.shape[0])
    E = r.sum(axis=1) - r[rows, m]
    q[rows, m] = (x[rows, m] + E).astype(np.float16)
    return (
        q.reshape(NPAIR, WB, 16, 16).transpose(0, 2, 1, 3).reshape(ROWS, W)
    )


def kernel(label: np.ndarray, energy: np.ndarray):
    from concourse import bass_utils

    store_e = _CACHE.get("store_e", STORE_E)
    nc = _get_nc()
    band = band_array()
    in_maps = []
    for i in range(B):
        lab_i = dither_label_fp16(
            np.ascontiguousarray(label[i, :, 160:320, :], dtype=np.float32).reshape(
                ROWS, W
            )
        )
        en_i = np.ascontiguousarray(energy[i]).astype(np.float16)
        in_maps.append({"label": lab_i, "energy": en_i, "band": band})

    res = bass_utils.run_bass_kernel_spmd(nc, in_maps, core_ids=list(range(B)))
    _CACHE["last_result"] = res

    att = np.stack([res.results[i]["att_out"].astype(np.float32) for i in range(B)])
    if store_e:
        e = np.stack([res.results[i]["e_out"].astype(np.float32) for i in range(B)])
    else:
        # z[p, j] = Z of row j*128+p
        z = np.stack([res.results[i]["z_out"].T.reshape(P) for i in range(B)])
        e = np.log(np.maximum(att, 1e-30)) + np.log(z)[:, :, None]
    return e, att
